# revision 1
# baseline (speedup 1.0000x reference)
"""Trainium2 Bass kernel for nn_MoELayer_67619965108245.

Dense MoE: B=64, N=55, D=512, E=8, L=4 SwiGLU layers per expert, H=2048.
Expert-parallel over 8 NeuronCores (one expert per core).

Layout: all activations live transposed in SBUF as [d_model, tokens]
("dT layout", tokens n-major: t = n*64 + b, N padded 55->56 so T=3584).
This makes every matmul in the SwiGLU chain transpose-free:
  gate^T[h,t] = sum_d Wg[d,h] * normed^T[d,t]      (lhsT = Wg natural)
  delta^T[d,t] = sum_h Wo[h,d] * gv^T[h,t]         (lhsT = Wo natural)
RMSNorm reductions over d (the partition dim) are done with ones-vector
matmuls on the PE; the per-token rstd row is broadcast back across
partitions with a K=1 ones matmul.

Each core computes its expert's full stack plus the complete router
softmax (all 8 logits per token), and returns
u_e[t] = expert_out_e[t] * softmax_weight_e[t]. Host just sums the 8
partial vectors. The RMSNorm scale is folded into Wg/Wv rows on host.

Matmuls run in bf16 (fp32 PSUM accumulation); the residual stream h
stays fp32 in SBUF.
"""

import numpy as np
import ml_dtypes

import concourse.bass as bass
import concourse.tile as tile
import concourse.mybir as mybir
from concourse.bass import ds, ts
from concourse.bass_utils import run_bass_kernel_spmd

B, N, D, E, L = 64, 55, 512, 8, 4
H = 4 * D
NP = 56          # padded node count
T = NP * B       # 3584 padded tokens, t = n*B + b
CH = 512         # token chunk (matmul free dim / PSUM bank)
NCH = T // CH    # 7
KD = D // 128    # 4 contraction chunks over d
KH = H // 128    # 16 contraction chunks over h
NPC = CH // B    # nodes per token chunk = 8
EPS = 1e-8

fp32 = mybir.dt.float32
bf16 = mybir.dt.bfloat16
bf16_np = ml_dtypes.bfloat16

# Walrus in this toolchain rejects instructions carrying more than one
# semaphore wait; Tile's final drain aggregates many. Split extras onto
# preceding same-engine NOPs (identical sync semantics).
_MAX_WAITS = 1


def _split_excess_waits(nc, max_waits=_MAX_WAITS):
    for f in nc.m.functions:
        for bb in f.blocks:
            insts = bb.instructions
            i = 0
            while i < len(insts):
                inst = insts[i]
                si = inst.sync_info
                if si is None or si.on_wait is None or len(si.on_wait) <= max_waits:
                    i += 1
                    continue
                waits = list(si.on_wait)
                keep, extra = waits[-max_waits:], waits[:-max_waits]
                nops = []
                for j in range(0, len(extra), max_waits):
                    nops.append(
                        mybir.InstNoOp(
                            name=f"{inst.name}_ws{j}",
                            engine=inst.engine,
                            ins=[],
                            outs=[],
                            sync_info=mybir.SyncInfo(
                                on_wait=extra[j : j + max_waits], on_update=[]
                            ),
                        )
                    )
                inst.sync_info = mybir.SyncInfo(
                    on_wait=keep, on_update=list(si.on_update or [])
                )
                for k, nop in enumerate(nops):
                    insts.insert(i + k, nop)
                i += len(nops) + 1


def build_bass():
    nc = bass.Bass("TRN2", target_bir_lowering=False, debug=False, num_devices=E)

    xT_d = nc.dram_tensor("xT", [KD, 128, T], fp32, kind="ExternalInput").ap()
    wg_d = nc.dram_tensor("wg", [L, 128, KD, H], bf16, kind="ExternalInput").ap()
    wv_d = nc.dram_tensor("wv", [L, 128, KD, H], bf16, kind="ExternalInput").ap()
    wo_d = nc.dram_tensor("wo", [L, 128, KH, D], bf16, kind="ExternalInput").ap()
    wr_d = nc.dram_tensor("wr", [128, NP, KD, E], fp32, kind="ExternalInput").ap()
    br_d = nc.dram_tensor("brt", [1, NP * E], fp32, kind="ExternalInput").ap()
    sel_d = nc.dram_tensor("sel", [E, 1], fp32, kind="ExternalInput").ap()
    wp_d = nc.dram_tensor("wp", [128, KD, 1], fp32, kind="ExternalInput").ap()
    bp_d = nc.dram_tensor("bps", [1, 1], fp32, kind="ExternalInput").ap()
    u_d = nc.dram_tensor("u", [1, T], fp32, kind="ExternalOutput").ap()

    with tile.TileContext(nc) as tc:
        from contextlib import ExitStack

        with ExitStack() as ctx:
            const = ctx.enter_context(tc.tile_pool(name="const", bufs=1))
            hp = ctx.enter_context(tc.tile_pool(name="hpool", bufs=1))
            wpg = ctx.enter_context(tc.tile_pool(name="wpg", bufs=1))
            wpv = ctx.enter_context(tc.tile_pool(name="wpv", bufs=1))
            wpo = ctx.enter_context(tc.tile_pool(name="wpo", bufs=1))
            nrm = ctx.enter_context(tc.tile_pool(name="nrm", bufs=4))
            sqp = ctx.enter_context(tc.tile_pool(name="sqp", bufs=2))
            gvp = ctx.enter_context(tc.tile_pool(name="gvp", bufs=1))
            silup = ctx.enter_context(tc.tile_pool(name="silup", bufs=3))
            smallp = ctx.enter_context(tc.tile_pool(name="smallp", bufs=4))
            routp = ctx.enter_context(tc.tile_pool(name="routp", bufs=2))
            outp = ctx.enter_context(tc.tile_pool(name="outp", bufs=2))
            pg = ctx.enter_context(tc.tile_pool(name="pg", bufs=2, space="PSUM"))
            pv = ctx.enter_context(tc.tile_pool(name="pv", bufs=2, space="PSUM"))
            pd = ctx.enter_context(tc.tile_pool(name="pd", bufs=2, space="PSUM"))
            pm = ctx.enter_context(tc.tile_pool(name="pm", bufs=2, space="PSUM"))

            # ---- constants ----
            ones_k_bf = const.tile([128, 1], bf16, name="ones_k_bf")
            nc.vector.memset(ones_k_bf, 1.0)
            ones_m_bf = const.tile([1, 128], bf16, name="ones_m_bf")
            nc.vector.memset(ones_m_bf, 1.0)
            ones_b_f = const.tile([1, B], fp32, name="ones_b_f")
            nc.vector.memset(ones_b_f, 1.0)
            ones_e_f = const.tile([E, 1], fp32, name="ones_e_f")
            nc.vector.memset(ones_e_f, 1.0)

            eps_sb = const.tile([1, 1], fp32, name="eps_sb")
            nc.vector.memset(eps_sb, EPS)
            sel_sb = const.tile([E, 1], fp32, name="sel_sb")
            nc.sync.dma_start(sel_sb[:], sel_d[:])
            br_sb = const.tile([1, NP * E], fp32, name="br_sb")
            nc.sync.dma_start(br_sb[:], br_d[:])
            wr_sb = const.tile([128, NP, KD, E], fp32, name="wr_sb")
            nc.sync.dma_start(wr_sb[:], wr_d[:])
            wp_sb = const.tile([128, KD, 1], fp32, name="wp_sb")
            nc.sync.dma_start(wp_sb[:], wp_d[:])
            bp_sb = const.tile([1, 1], fp32, name="bp_sb")
            nc.sync.dma_start(bp_sb[:], bp_d[:])
            w_sb = const.tile([1, T], fp32, name="w_sb")  # router weight row

            # ---- residual state (fp32, dT layout) ----
            h = []
            for k in range(KD):
                hk = hp.tile([128, T], fp32, name=f"h{k}", tag=f"h{k}")
                nc.sync.dma_start(hk[:], xT_d[k])
                h.append(hk)

            # ---- router: all-E logits, softmax, own-expert weight row ----
            for c in range(NCH):
                cs = ds(c * CH, CH)
                lg = pm.tile([128, CH], fp32, name=f"lg{c}", tag="pm")
                for ni in range(NPC):
                    n = c * NPC + ni
                    off = ni * B
                    for k in range(KD):
                        nc.tensor.matmul(
                            lg[0:E, ds(off, B)],
                            wr_sb[:, n, k, :],
                            h[k][:, ds(n * B, B)],
                            start=(k == 0),
                            stop=False,
                        )
                    nc.tensor.matmul(
                        lg[0:E, ds(off, B)],
                        br_sb[0:1, ds(n * E, E)],
                        ones_b_f[:],
                        start=False,
                        stop=True,
                    )
                expc = routp.tile([E, CH], fp32, name=f"expc{c}", tag="expc")
                nc.scalar.activation(
                    expc[:], lg[0:E, :], mybir.ActivationFunctionType.Exp
                )
                den = pm.tile([128, CH], fp32, name=f"den{c}", tag="pm")
                nc.tensor.matmul(
                    den[0:1, :], ones_e_f[:], expc[:], start=True, stop=True
                )
                num = pd.tile([128, CH], fp32, name=f"num{c}", tag="pd")
                nc.tensor.matmul(
                    num[0:1, :], sel_sb[:], expc[:], start=True, stop=True
                )
                rden = smallp.tile([1, CH], fp32, name=f"rden{c}", tag="rden")
                nc.vector.reciprocal(rden[:], den[0:1, :])
                nc.vector.tensor_mul(w_sb[:, cs], num[0:1, :], rden[:])

            # ---- expert MLP stack ----
            for l in range(L):
                wg_sb = wpg.tile([128, KD, H], bf16, name=f"wg{l}", tag="wg")
                nc.sync.dma_start(wg_sb[:], wg_d[l])
                wv_sb = wpv.tile([128, KD, H], bf16, name=f"wv{l}", tag="wv")
                nc.sync.dma_start(wv_sb[:], wv_d[l])
                wo_sb = wpo.tile([128, KH, D], bf16, name=f"wo{l}", tag="wo")
                nc.sync.dma_start(wo_sb[:], wo_d[l])

                normed = []
                # --- rmsnorm phase (all chunks) ---
                for c in range(NCH):
                    cs = ds(c * CH, CH)
                    sq = sqp.tile([128, KD, CH], bf16, name=f"sq{l}_{c}", tag="sq")
                    for k in range(KD):
                        nc.vector.tensor_mul(sq[:, k, :], h[k][:, cs], h[k][:, cs])
                    msq = pm.tile([128, CH], fp32, name=f"ms{l}_{c}", tag="pm")
                    for k in range(KD):
                        nc.tensor.matmul(
                            msq[0:1, :],
                            ones_k_bf[:],
                            sq[:, k, :],
                            start=(k == 0),
                            stop=(k == KD - 1),
                        )
                    # std = sqrt(mean + eps); rstd = 1/std (bf16 is enough:
                    # normed is rounded to bf16 right after anyway)
                    std = smallp.tile([1, CH], fp32, name=f"std{l}_{c}", tag="std")
                    nc.scalar.activation(
                        std[:],
                        msq[0:1, :],
                        mybir.ActivationFunctionType.Sqrt,
                        bias=eps_sb[:],
                        scale=1.0 / D,
                    )
                    rstd = smallp.tile([1, CH], bf16, name=f"rstd{l}_{c}", tag="rstd")
                    with nc.allow_low_precision(
                        reason="rstd feeds bf16 normed; bf16 rstd is free precision-wise"
                    ):
                        nc.vector.reciprocal(rstd[:], std[:])
                    bc = pm.tile([128, CH], fp32, name=f"bc{l}_{c}", tag="pm")
                    nc.tensor.matmul(
                        bc[:], ones_m_bf[:], rstd[:], start=True, stop=True
                    )
                    nt = nrm.tile([128, KD, CH], bf16, name=f"nt{l}_{c}", tag="nt")
                    for k in range(KD):
                        nc.vector.tensor_mul(nt[:, k, :], h[k][:, cs], bc[:])
                    normed.append(nt)

                # --- SwiGLU phase ---
                for c in range(NCH):
                    cs = ds(c * CH, CH)
                    nt = normed[c]
                    gv = gvp.tile([128, KH, CH], bf16, name=f"gv{l}_{c}", tag="gv")
                    for j in range(KH):
                        gps = pg.tile([128, CH], fp32, name=f"g{l}_{c}_{j}", tag="pg")
                        vps = pv.tile([128, CH], fp32, name=f"v{l}_{c}_{j}", tag="pv")
                        for k in range(KD):
                            nc.tensor.matmul(
                                gps[:],
                                wg_sb[:, k, ts(j, 128)],
                                nt[:, k, :],
                                start=(k == 0),
                                stop=(k == KD - 1),
                            )
                        for k in range(KD):
                            nc.tensor.matmul(
                                vps[:],
                                wv_sb[:, k, ts(j, 128)],
                                nt[:, k, :],
                                start=(k == 0),
                                stop=(k == KD - 1),
                            )
                        sil = silup.tile([128, CH], bf16, name=f"sl{l}_{c}_{j}", tag="sil")
                        nc.scalar.activation(
                            sil[:], gps[:], mybir.ActivationFunctionType.Silu
                        )
                        nc.vector.tensor_mul(gv[:, j, :], sil[:], vps[:])
                    for i in range(KD):
                        dps = pd.tile([128, CH], fp32, name=f"d{l}_{c}_{i}", tag="pd")
                        for j in range(KH):
                            nc.tensor.matmul(
                                dps[:],
                                wo_sb[:, j, ts(i, 128)],
                                gv[:, j, :],
                                start=(j == 0),
                                stop=(j == KH - 1),
                            )
                        nc.vector.tensor_add(h[i][:, cs], h[i][:, cs], dps[:])

                    if l == L - 1:
                        # final projection + router weighting for this chunk
                        eo = pm.tile([128, CH], fp32, name=f"eo{c}", tag="pm")
                        for k in range(KD):
                            nc.tensor.matmul(
                                eo[0:1, :],
                                wp_sb[:, k, :],
                                h[k][:, cs],
                                start=(k == 0),
                                stop=(k == KD - 1),
                            )
                        eos = outp.tile([1, CH], fp32, name=f"eos{c}", tag="eos")
                        nc.scalar.activation(
                            eos[:],
                            eo[0:1, :],
                            mybir.ActivationFunctionType.Identity,
                            bias=bp_sb[:],
                        )
                        us = outp.tile([1, CH], fp32, name=f"us{c}", tag="us")
                        nc.vector.tensor_mul(us[:], eos[:], w_sb[:, cs])
                        nc.sync.dma_start(u_d[0:1, cs], us[:])

    _split_excess_waits(nc)
    return nc


_CACHE = {}


def _get_nc():
    if "nc" not in _CACHE:
        _CACHE["nc"] = build_bass()
    return _CACHE["nc"]


def _prep_inputs(x, scale, Wg, Wv, Wo, Wp, bp, Wr, br):
    x = np.asarray(x, np.float32)
    scale = np.asarray(scale, np.float32)
    Wg = np.asarray(Wg, np.float32)
    Wv = np.asarray(Wv, np.float32)
    Wo = np.asarray(Wo, np.float32)
    Wp = np.asarray(Wp, np.float32)
    bp = np.asarray(bp, np.float32)
    Wr = np.asarray(Wr, np.float32)
    br = np.asarray(br, np.float32)

    # xT: [d, n, b] padded -> [KD, 128, T]
    xt = np.zeros((D, NP, B), np.float32)
    xt[:, :N, :] = x.transpose(2, 1, 0)
    xT = np.ascontiguousarray(xt.reshape(KD, 128, T))

    # router tensors (shared by all cores)
    wr_full = np.zeros((NP, E, D), np.float32)
    wr_full[:N] = Wr
    wr_prep = np.ascontiguousarray(
        wr_full.transpose(2, 0, 1).reshape(KD, 128, NP, E).transpose(1, 2, 0, 3)
    )
    br_full = np.zeros((NP, E), np.float32)
    br_full[:N] = br
    br_prep = np.ascontiguousarray(br_full.reshape(1, NP * E))

    # fold RMSNorm scale into Wg/Wv rows: (L, E, D, H)
    wg_eff = Wg * scale[:, :, :, None]
    wv_eff = Wv * scale[:, :, :, None]

    in_maps = []
    for e in range(E):
        wg_p = np.ascontiguousarray(
            wg_eff[:, e].reshape(L, KD, 128, H).transpose(0, 2, 1, 3)
        ).astype(bf16_np)
        wv_p = np.ascontiguousarray(
            wv_eff[:, e].reshape(L, KD, 128, H).transpose(0, 2, 1, 3)
        ).astype(bf16_np)
        wo_p = np.ascontiguousarray(
            Wo[:, e].reshape(L, KH, 128, D).transpose(0, 2, 1, 3)
        ).astype(bf16_np)
        wp_p = np.ascontiguousarray(
            Wp[e].reshape(KD, 128, 1).transpose(1, 0, 2)
        )
        sel = np.zeros((E, 1), np.float32)
        sel[e, 0] = 1.0
        in_maps.append(
            {
                "xT": xT,
                "wg": wg_p,
                "wv": wv_p,
                "wo": wo_p,
                "wr": wr_prep,
                "brt": br_prep,
                "sel": sel,
                "wp": wp_p,
                "bps": np.array([[bp[e]]], np.float32),
            }
        )
    return in_maps


def _combine(results):
    u = np.zeros(T, np.float64)
    for r in results:
        u += r["u"].reshape(T).astype(np.float64)
    return np.ascontiguousarray(u.reshape(NP, B)[:N, :].T).astype(np.float32)


def kernel(x, scale, Wg, Wv, Wo, Wp, bp, Wr, br):
    nc = _get_nc()
    in_maps = _prep_inputs(x, scale, Wg, Wv, Wo, Wp, bp, Wr, br)
    res = run_bass_kernel_spmd(nc, in_maps, list(range(E)))
    return _combine(res.results)



# revision 26
# speedup vs baseline: 1.8915x; 1.8915x over previous
"""Trainium2 Bass kernel for nn_MoELayer_67619965108245.

Dense MoE: B=64, N=55, D=512, E=8, L=4 SwiGLU layers per expert, H=2048.
Expert-parallel over 8 NeuronCores (one expert per core).

Layout: all activations live transposed in SBUF as [d_model, tokens]
("dT layout", tokens n-major: t = n*64 + b, N padded 55->56 so T=3584).
This makes every matmul in the SwiGLU chain transpose-free:
  gate^T[h,t] = sum_d Wg[d,h] * normed^T[d,t]      (lhsT = Wg natural)
  delta^T[d,t] = sum_h Wo[h,d] * gv^T[h,t]         (lhsT = Wo natural)
RMSNorm reductions over d (the partition dim) are done with ones-vector
matmuls on the PE; the per-token rstd row is broadcast back across
partitions with a K=1 ones matmul.

The three big SwiGLU matmuls run in fp8-e4m3 with perf_mode=DoubleRow
(2 fp8 weights per PE cell, 2 MACs/cycle -> ~2x bf16 matmul
throughput). Scaling keeps everything in e4m3's good range with zero
extra instructions: x is pre-scaled by SH=2048 on host and the residual
stream h stays at 2048x true scale all 4 layers (RMSNorm is
scale-invariant, so normed is always unit-scale); Wg*SG, Wv*SV, Wo*SO
are folded on host with SV*SO == SH so the Wo PSUM lands back at h's
scale and the residual add needs no rescale. The silu reads its PSUM
with scale=1/SG; the gv product keeps the val PSUM's SV factor as its
fp8 range boost. Wr and Wp are pre-divided by SH on host.

Each core computes its expert's full stack plus the complete router
softmax (all 8 logits per token, in bf16), and returns
u_e[t] = expert_out_e[t] * softmax_weight_e[t]. Host just sums the 8
partial vectors. The RMSNorm scale is folded into Wg/Wv rows on host.
"""

import numpy as np
import ml_dtypes

import concourse.bass as bass
import concourse.tile as tile
import concourse.mybir as mybir
from concourse.bass import ds, ts
from concourse.bass_utils import run_bass_kernel_spmd

B, N, D, E, L = 64, 55, 512, 8, 4
H = 4 * D
T = N * B        # 3520 tokens, t = n*B + b (no padding)
CH = 512         # token chunk (matmul free dim / PSUM bank)
NCH = (T + CH - 1) // CH            # 7 (last chunk short: 448)
CHS = [min(CH, T - c * CH) for c in range(NCH)]  # per-chunk sizes
KD = D // 128    # 4 contraction chunks over d
KH = H // 128    # 16 contraction chunks over h
EPS = 1e-8

# fp8 scaling (see module docstring); SV * SO must equal SH
SH = 2048.0
SG = 64.0
SV = 32.0
SO = 64.0

# Software-interleaved DoubleRow: weights pre-interleaved on host so the
# PE weight load reads contiguously instead of doing the DoubleRow
# hardware interleave read.
SWI = False

fp32 = mybir.dt.float32
bf16 = mybir.dt.bfloat16
fp8 = mybir.dt.float8e4
bf16_np = ml_dtypes.bfloat16
fp8_np = ml_dtypes.float8_e4m3
DR = mybir.MatmulPerfMode.DoubleRow

# Walrus in this toolchain rejects instructions carrying more than one
# semaphore wait; Tile's final drain aggregates many. Split extras onto
# preceding same-engine NOPs (identical sync semantics).
_MAX_WAITS = 1


def _split_excess_waits(nc, max_waits=_MAX_WAITS):
    for f in nc.m.functions:
        for bb in f.blocks:
            insts = bb.instructions
            i = 0
            while i < len(insts):
                inst = insts[i]
                si = inst.sync_info
                if si is None or si.on_wait is None or len(si.on_wait) <= max_waits:
                    i += 1
                    continue
                waits = list(si.on_wait)
                keep, extra = waits[-max_waits:], waits[:-max_waits]
                nops = []
                for j in range(0, len(extra), max_waits):
                    nops.append(
                        mybir.InstNoOp(
                            name=f"{inst.name}_ws{j}",
                            engine=inst.engine,
                            ins=[],
                            outs=[],
                            sync_info=mybir.SyncInfo(
                                on_wait=extra[j : j + max_waits], on_update=[]
                            ),
                        )
                    )
                inst.sync_info = mybir.SyncInfo(
                    on_wait=keep, on_update=list(si.on_update or [])
                )
                for k, nop in enumerate(nops):
                    insts.insert(i + k, nop)
                i += len(nops) + 1


def build_bass(split_waits=True):
    nc = bass.Bass("TRN2", target_bir_lowering=False, debug=False, num_devices=E)

    xT_d = nc.dram_tensor("xT", [KD, 128, T], fp32, kind="ExternalInput").ap()
    if SWI:
        wg_d = nc.dram_tensor(
            "wg", [L, 128, KD // 2, KH, 256], fp8, kind="ExternalInput"
        ).ap()
        wv_d = nc.dram_tensor(
            "wv", [L, 128, KD // 2, KH, 256], fp8, kind="ExternalInput"
        ).ap()
        wo_d = nc.dram_tensor(
            "wo", [L, 128, KH // 2, KD, 256], fp8, kind="ExternalInput"
        ).ap()
    else:
        wg_d = nc.dram_tensor("wg", [L, 128, KD, H], fp8, kind="ExternalInput").ap()
        wv_d = nc.dram_tensor("wv", [L, 128, KD, H], fp8, kind="ExternalInput").ap()
        wo_d = nc.dram_tensor("wo", [L, 128, KH, D], fp8, kind="ExternalInput").ap()
    wr_d = nc.dram_tensor("wr", [128, N, KD, E], bf16, kind="ExternalInput").ap()
    br_d = nc.dram_tensor("brt", [1, N * E], bf16, kind="ExternalInput").ap()
    sel_d = nc.dram_tensor("sel", [E, 1], bf16, kind="ExternalInput").ap()
    wp_d = nc.dram_tensor("wp", [128, KD, 1], bf16, kind="ExternalInput").ap()
    bp_d = nc.dram_tensor("bps", [1, 1], fp32, kind="ExternalInput").ap()
    u_d = nc.dram_tensor("u", [1, T], fp32, kind="ExternalOutput").ap()

    with tile.TileContext(nc) as tc:
        from contextlib import ExitStack

        with ExitStack() as ctx:
            const = ctx.enter_context(tc.tile_pool(name="const", bufs=1))
            hp = ctx.enter_context(tc.tile_pool(name="hpool", bufs=1))
            wpg = ctx.enter_context(tc.tile_pool(name="wpg", bufs=2))
            wpv = ctx.enter_context(tc.tile_pool(name="wpv", bufs=2))
            wpo = ctx.enter_context(tc.tile_pool(name="wpo", bufs=2))
            nrm = ctx.enter_context(tc.tile_pool(name="nrm", bufs=4))
            sqp = ctx.enter_context(tc.tile_pool(name="sqp", bufs=2))
            gvp = ctx.enter_context(tc.tile_pool(name="gvp", bufs=2))
            silup = ctx.enter_context(tc.tile_pool(name="silup", bufs=4))
            smallp = ctx.enter_context(tc.tile_pool(name="smallp", bufs=4))
            routp = ctx.enter_context(tc.tile_pool(name="routp", bufs=2))
            outp = ctx.enter_context(tc.tile_pool(name="outp", bufs=2))
            pg = ctx.enter_context(tc.tile_pool(name="pg", bufs=2, space="PSUM"))
            pv = ctx.enter_context(tc.tile_pool(name="pv", bufs=2, space="PSUM"))
            pd = ctx.enter_context(tc.tile_pool(name="pd", bufs=2, space="PSUM"))
            pm = ctx.enter_context(tc.tile_pool(name="pm", bufs=2, space="PSUM"))

            # ---- constants ----
            ones_k_bf = const.tile([128, 1], bf16, name="ones_k_bf")
            nc.vector.memset(ones_k_bf, 1.0)
            ones_m_bf = const.tile([1, 128], bf16, name="ones_m_bf")
            nc.vector.memset(ones_m_bf, 1.0)
            ones_b_bf = const.tile([1, B], bf16, name="ones_b_bf")
            nc.vector.memset(ones_b_bf, 1.0)
            ones_e_bf = const.tile([E, 1], bf16, name="ones_e_bf")
            nc.vector.memset(ones_e_bf, 1.0)

            eps_sb = const.tile([1, 1], fp32, name="eps_sb")
            nc.vector.memset(eps_sb, EPS)
            sel_sb = const.tile([E, 1], bf16, name="sel_sb")
            nc.sync.dma_start(sel_sb[:], sel_d[:])
            br_sb = const.tile([1, N * E], bf16, name="br_sb")
            nc.sync.dma_start(br_sb[:], br_d[:])
            wr_sb = const.tile([128, N, KD, E], bf16, name="wr_sb")
            nc.sync.dma_start(wr_sb[:], wr_d[:])
            wp_sb = const.tile([128, KD, 1], bf16, name="wp_sb")
            nc.sync.dma_start(wp_sb[:], wp_d[:])
            bp_sb = const.tile([1, 1], fp32, name="bp_sb")
            nc.sync.dma_start(bp_sb[:], bp_d[:])
            w_sb = const.tile([1, T], fp32, name="w_sb")  # router weight row

            # ---- residual state (fp32 at SH x true scale, dT layout) ----
            h = []
            for k in range(KD):
                hk = hp.tile([128, T], fp32, name=f"h{k}", tag=f"h{k}")
                h.append(hk)
            # per-chunk loads so chunk-0 compute starts after ~256KB, not 7MB
            for c in range(NCH):
                cs = ds(c * CH, CHS[c])
                for k in range(KD):
                    nc.sync.dma_start(h[k][:, cs], xT_d[k][:, cs])

            # ---- router: all-E logits (bf16), softmax, own weight row.
            # Emitted per chunk, interleaved with layer-0 rmsnorm so the
            # small router matmuls fill PE gaps instead of forming a
            # serial phase at kernel start.
            def emit_router(c):
                sz = CHS[c]
                cs = ds(c * CH, sz)
                xb = routp.tile([128, KD, CH], bf16, name=f"xb{c}", tag="xb")
                for k in range(KD):
                    nc.vector.tensor_copy(xb[:, k, :sz], h[k][:, cs])
                lg = pm.tile([128, CH], fp32, name=f"lg{c}", tag="pm")
                for ni in range(sz // B):
                    n = (c * CH) // B + ni
                    off = ni * B
                    for k in range(KD):
                        nc.tensor.matmul(
                            lg[0:E, ds(off, B)],
                            wr_sb[:, n, k, :],
                            xb[:, k, ds(off, B)],
                            start=(k == 0),
                            stop=False,
                        )
                    nc.tensor.matmul(
                        lg[0:E, ds(off, B)],
                        br_sb[0:1, ds(n * E, E)],
                        ones_b_bf[:],
                        start=False,
                        stop=True,
                    )
                expc = routp.tile([E, CH], bf16, name=f"expc{c}", tag="expc")
                with nc.allow_low_precision(
                    reason="bf16 exp perturbs num and den together; the ratio "
                    "keeps softmax weights to ~0.4% which is noise here"
                ):
                    nc.scalar.activation(
                        expc[:, :sz], lg[0:E, :sz], mybir.ActivationFunctionType.Exp
                    )
                den = pm.tile([128, CH], fp32, name=f"den{c}", tag="pm")
                nc.tensor.matmul(
                    den[0:1, :sz], ones_e_bf[:], expc[:, :sz], start=True, stop=True
                )
                num = pd.tile([128, CH], fp32, name=f"num{c}", tag="pd")
                nc.tensor.matmul(
                    num[0:1, :sz], sel_sb[:], expc[:, :sz], start=True, stop=True
                )
                rden = smallp.tile([1, CH], fp32, name=f"rden{c}", tag="rden")
                nc.vector.reciprocal(rden[:, :sz], den[0:1, :sz])
                nc.vector.tensor_mul(w_sb[:, cs], num[0:1, :sz], rden[:, :sz])

            # ---- expert MLP stack ----
            DRM = mybir.MatmulPerfMode.DoubleRowSwInterleave if SWI else DR
            for l in range(L):
                if SWI:
                    wg_sb = wpg.tile([128, KD // 2, KH, 256], fp8, name=f"wg{l}", tag="wg")
                    wv_sb = wpv.tile([128, KD // 2, KH, 256], fp8, name=f"wv{l}", tag="wv")
                    wo_sb = wpo.tile([128, KH // 2, KD, 256], fp8, name=f"wo{l}", tag="wo")
                else:
                    wg_sb = wpg.tile([128, KD, H], fp8, name=f"wg{l}", tag="wg")
                    wv_sb = wpv.tile([128, KD, H], fp8, name=f"wv{l}", tag="wv")
                    wo_sb = wpo.tile([128, KH, D], fp8, name=f"wo{l}", tag="wo")
                # split weight loads so the first j-blocks land early
                if SWI:
                    for g in range(4):
                        nc.sync.dma_start(
                            wg_sb[:, :, ds(4 * g, 4), :], wg_d[l][:, :, ds(4 * g, 4), :]
                        )
                        nc.sync.dma_start(
                            wv_sb[:, :, ds(4 * g, 4), :], wv_d[l][:, :, ds(4 * g, 4), :]
                        )
                        nc.sync.dma_start(
                            wo_sb[:, ds(2 * g, 2), :, :], wo_d[l][:, ds(2 * g, 2), :, :]
                        )
                else:
                    nc.sync.dma_start(wg_sb[:], wg_d[l])
                    nc.sync.dma_start(wv_sb[:], wv_d[l])
                    nc.sync.dma_start(wo_sb[:], wo_d[l])

                normed = {}
                # --- rmsnorm, emitted two chunks ahead of SwiGLU use so
                # its small PE matmuls interleave between DR bursts instead
                # of bunching (and stalling) at the layer boundary ---
                def emit_norm(c):
                    sz = CHS[c]
                    cs = ds(c * CH, sz)
                    if l == 0:
                        emit_router(c)
                    sq = sqp.tile([128, KD, CH], bf16, name=f"sq{l}_{c}", tag="sq")
                    for k in range(KD):
                        # ACT, not DVE: DVE is the busier engine in this phase
                        nc.scalar.activation(
                            sq[:, k, :sz],
                            h[k][:, cs],
                            mybir.ActivationFunctionType.Square,
                        )
                    msq = pm.tile([128, CH], fp32, name=f"ms{l}_{c}", tag="pm")
                    for k in range(KD):
                        nc.tensor.matmul(
                            msq[0:1, :sz],
                            ones_k_bf[:],
                            sq[:, k, :sz],
                            start=(k == 0),
                            stop=(k == KD - 1),
                        )
                    # std = sqrt(mean + eps); rstd = 1/std (bf16 is enough:
                    # normed is rounded to fp8 right after anyway)
                    std = smallp.tile([1, CH], fp32, name=f"std{l}_{c}", tag="std")
                    nc.scalar.activation(
                        std[:, :sz],
                        msq[0:1, :sz],
                        mybir.ActivationFunctionType.Sqrt,
                        bias=eps_sb[:],
                        scale=1.0 / D,
                    )
                    rstd = smallp.tile([1, CH], bf16, name=f"rstd{l}_{c}", tag="rstd")
                    with nc.allow_low_precision(
                        reason="rstd feeds fp8 normed; bf16 rstd is free precision-wise"
                    ):
                        nc.vector.reciprocal(rstd[:, :sz], std[:, :sz])
                    bc = pm.tile([128, CH], fp32, name=f"bc{l}_{c}", tag="pm")
                    nc.tensor.matmul(
                        bc[:, :sz], ones_m_bf[:], rstd[:, :sz], start=True, stop=True
                    )
                    nt = nrm.tile([128, KD, CH], fp8, name=f"nt{l}_{c}", tag="nt")
                    with nc.allow_low_precision(
                        reason="fp8 normed feeds the DoubleRow matmuls; unit "
                        "scale sits in e4m3's good range"
                    ):
                        for k in range(KD):
                            nc.vector.tensor_mul(nt[:, k, :sz], h[k][:, cs], bc[:, :sz])
                    normed[c] = nt

                # --- SwiGLU phase (fp8 DoubleRow matmuls) ---
                # Down-projection matmuls for chunk c are emitted during
                # chunk c+1's gate/val phase: the PE is in-order, so putting
                # a full chunk of independent gate/val work between the last
                # gv write and the down matmuls that read it removes the
                # per-chunk PE stall on the silu+mul tail.
                gv_tiles = {}

                def emit_down(c):
                    sz = CHS[c]
                    cs_ = ds(c * CH, sz)
                    gv_ = gv_tiles.pop(c)
                    if l == L - 1:
                        hbc = outp.tile([128, KD, CH], bf16, name=f"hb{c}", tag="hb")
                    for i in range(KD):
                        dps = pd.tile([128, CH], fp32, name=f"d{l}_{c}_{i}", tag="pd")
                        for j in range(KH // 2):
                            nc.tensor.matmul(
                                dps[:, :sz],
                                wo_sb[:, j, i, :] if SWI
                                else wo_sb[:, ds(2 * j, 2), ts(i, 128)],
                                gv_[:, ds(2 * j, 2), :sz],
                                start=(j == 0),
                                stop=(j == KH // 2 - 1),
                                perf_mode=DRM,
                            )
                        if l < L - 1:
                            nc.vector.tensor_add(
                                h[i][:, cs_], h[i][:, cs_], dps[:, :sz]
                            )
                        else:
                            with nc.allow_low_precision(
                                reason="final projection tolerates bf16 residual"
                            ):
                                nc.vector.tensor_add(
                                    hbc[:, i, :sz], h[i][:, cs_], dps[:, :sz]
                                )
                    if l == L - 1:
                        # final projection + router weighting for this chunk
                        eo = pm.tile([128, CH], fp32, name=f"eo{c}", tag="pm")
                        for k in range(KD):
                            nc.tensor.matmul(
                                eo[0:1, :sz],
                                wp_sb[:, k, :],
                                hbc[:, k, :sz],
                                start=(k == 0),
                                stop=(k == KD - 1),
                            )
                        eos = outp.tile([1, CH], fp32, name=f"eos{c}", tag="eos")
                        nc.scalar.activation(
                            eos[:, :sz],
                            eo[0:1, :sz],
                            mybir.ActivationFunctionType.Identity,
                            bias=bp_sb[:],
                        )
                        us = outp.tile([1, CH], fp32, name=f"us{c}", tag="us")
                        nc.vector.tensor_mul(us[:, :sz], eos[:, :sz], w_sb[:, cs_])
                        nc.sync.dma_start(u_d[0:1, cs_], us[:, :sz])

                emit_norm(0)
                if NCH > 1:
                    emit_norm(1)
                for c in range(NCH):
                    if c + 2 < NCH:
                        emit_norm(c + 2)
                    sz = CHS[c]
                    nt = normed.pop(c)
                    gv = gvp.tile([128, KH, CH], fp8, name=f"gv{l}_{c}", tag="gv")
                    gv_tiles[c] = gv
                    for j in range(KH):
                        gps = pg.tile([128, CH], fp32, name=f"g{l}_{c}_{j}", tag="pg")
                        vps = pv.tile([128, CH], fp32, name=f"v{l}_{c}_{j}", tag="pv")
                        for k in range(KD // 2):
                            nc.tensor.matmul(
                                gps[:, :sz],
                                wg_sb[:, k, j, :] if SWI
                                else wg_sb[:, ds(2 * k, 2), ts(j, 128)],
                                nt[:, ds(2 * k, 2), :sz],
                                start=(k == 0),
                                stop=(k == KD // 2 - 1),
                                perf_mode=DRM,
                            )
                        for k in range(KD // 2):
                            nc.tensor.matmul(
                                vps[:, :sz],
                                wv_sb[:, k, j, :] if SWI
                                else wv_sb[:, ds(2 * k, 2), ts(j, 128)],
                                nt[:, ds(2 * k, 2), :sz],
                                start=(k == 0),
                                stop=(k == KD // 2 - 1),
                                perf_mode=DRM,
                            )
                        sil = silup.tile([128, CH], bf16, name=f"sl{l}_{c}_{j}", tag="sil")
                        nc.scalar.activation(
                            sil[:, :sz],
                            gps[:, :sz],
                            mybir.ActivationFunctionType.Silu,
                            scale=1.0 / SG,
                        )
                        with nc.allow_low_precision(
                            reason="gv carries the val PSUM's SV factor, which "
                            "centers it in e4m3's range for the Wo matmul"
                        ):
                            nc.vector.tensor_mul(
                                gv[:, j, :sz], sil[:, :sz], vps[:, :sz]
                            )
                    if c >= 1:
                        emit_down(c - 1)
                emit_down(NCH - 1)

    if split_waits:
        _split_excess_waits(nc)
    return nc


_CACHE = {}


def _get_nc():
    if "nc" not in _CACHE:
        _CACHE["nc"] = build_bass()
    return _CACHE["nc"]


def _prep_inputs(x, scale, Wg, Wv, Wo, Wp, bp, Wr, br):
    x = np.asarray(x, np.float32)
    scale = np.asarray(scale, np.float32)
    Wg = np.asarray(Wg, np.float32)
    Wv = np.asarray(Wv, np.float32)
    Wo = np.asarray(Wo, np.float32)
    Wp = np.asarray(Wp, np.float32)
    bp = np.asarray(bp, np.float32)
    Wr = np.asarray(Wr, np.float32)
    br = np.asarray(br, np.float32)

    # xT: [d, n, b] (no padding), pre-scaled by SH -> [KD, 128, T]
    xt = x.transpose(2, 1, 0) * SH
    xT = np.ascontiguousarray(xt.reshape(KD, 128, T))

    # router tensors (shared by all cores); Wr pre-divided by SH
    wr_prep = np.ascontiguousarray(
        (Wr / SH).transpose(2, 0, 1).reshape(KD, 128, N, E).transpose(1, 2, 0, 3)
    ).astype(bf16_np)
    br_prep = np.ascontiguousarray(br.reshape(1, N * E)).astype(bf16_np)

    # fold RMSNorm scale into Wg/Wv rows: (L, E, D, H); fp8 range scales
    def q8(a):
        return np.clip(a, -240.0, 240.0).astype(fp8_np)

    wg_eff = Wg * scale[:, :, :, None] * SG
    wv_eff = Wv * scale[:, :, :, None] * SV
    wo_eff = Wo * SO

    def swi(w, kt, blocks):
        # (L, 128, kt, blocks*128) -> pre-interleaved DoubleRowSwInterleave
        # layout (L, 128, kt//2, blocks, 256): per 128-column block, pairs
        # (A[127-c], B[127-c]) of the two k-tiles, columns reversed.
        w5 = w.reshape(L, 128, kt // 2, 2, blocks, 128)
        rev = w5[..., ::-1]
        return np.ascontiguousarray(
            rev.transpose(0, 1, 2, 4, 5, 3).reshape(L, 128, kt // 2, blocks, 256)
        )

    in_maps = []
    for e in range(E):
        wg_p = q8(np.ascontiguousarray(
            wg_eff[:, e].reshape(L, KD, 128, H).transpose(0, 2, 1, 3)
        ))
        wv_p = q8(np.ascontiguousarray(
            wv_eff[:, e].reshape(L, KD, 128, H).transpose(0, 2, 1, 3)
        ))
        wo_p = q8(np.ascontiguousarray(
            wo_eff[:, e].reshape(L, KH, 128, D).transpose(0, 2, 1, 3)
        ))
        if SWI:
            wg_p = swi(wg_p, KD, KH)
            wv_p = swi(wv_p, KD, KH)
            wo_p = swi(wo_p, KH, KD)
        wp_p = np.ascontiguousarray(
            (Wp[e] / SH).reshape(KD, 128, 1).transpose(1, 0, 2)
        ).astype(bf16_np)
        sel = np.zeros((E, 1), bf16_np)
        sel[e, 0] = 1.0
        in_maps.append(
            {
                "xT": xT,
                "wg": wg_p,
                "wv": wv_p,
                "wo": wo_p,
                "wr": wr_prep,
                "brt": br_prep,
                "sel": sel,
                "wp": wp_p,
                "bps": np.array([[bp[e]]], np.float32),
            }
        )
    return in_maps


def _combine(results):
    u = np.zeros(T, np.float64)
    for r in results:
        u += r["u"].reshape(T).astype(np.float64)
    return np.ascontiguousarray(u.reshape(N, B).T).astype(np.float32)


def kernel(x, scale, Wg, Wv, Wo, Wp, bp, Wr, br):
    nc = _get_nc()
    in_maps = _prep_inputs(x, scale, Wg, Wv, Wo, Wp, bp, Wr, br)
    res = run_bass_kernel_spmd(nc, in_maps, list(range(E)))
    return _combine(res.results)


# revision 28
# speedup vs baseline: 3.6142x; 1.9108x over previous
"""Trainium2 Bass kernel for nn_MoELayer_67619965108245.

Dense MoE: B=64, N=55, D=512, E=8, L=4 SwiGLU layers per expert, H=2048.
Expert-parallel over 8 NeuronCores (one expert per core).

Layout: all activations live transposed in SBUF as [d_model, tokens]
("dT layout", tokens n-major: t = n*64 + b, N padded 55->56 so T=3584).
This makes every matmul in the SwiGLU chain transpose-free:
  gate^T[h,t] = sum_d Wg[d,h] * normed^T[d,t]      (lhsT = Wg natural)
  delta^T[d,t] = sum_h Wo[h,d] * gv^T[h,t]         (lhsT = Wo natural)
RMSNorm reductions over d (the partition dim) are done with ones-vector
matmuls on the PE; the per-token rstd row is broadcast back across
partitions with a K=1 ones matmul.

The three big SwiGLU matmuls run in fp8-e4m3 with perf_mode=DoubleRow
(2 fp8 weights per PE cell, 2 MACs/cycle -> ~2x bf16 matmul
throughput). Scaling keeps everything in e4m3's good range with zero
extra instructions: x is pre-scaled by SH=2048 on host and the residual
stream h stays at 2048x true scale all 4 layers (RMSNorm is
scale-invariant, so normed is always unit-scale); Wg*SG, Wv*SV, Wo*SO
are folded on host with SV*SO == SH so the Wo PSUM lands back at h's
scale and the residual add needs no rescale. The silu reads its PSUM
with scale=1/SG; the gv product keeps the val PSUM's SV factor as its
fp8 range boost. Wr and Wp are pre-divided by SH on host.

Each core computes its expert's full stack plus the complete router
softmax (all 8 logits per token, in bf16), and returns
u_e[t] = expert_out_e[t] * softmax_weight_e[t]. Host just sums the 8
partial vectors. The RMSNorm scale is folded into Wg/Wv rows on host.
"""

import numpy as np
import ml_dtypes

import concourse.bass as bass
import concourse.tile as tile
import concourse.mybir as mybir
from concourse.bass import ds, ts
from concourse.bass_utils import run_bass_kernel_spmd

B, N, D, E, L = 64, 55, 512, 8, 4
H = 4 * D
T = N * B        # 3520 tokens, t = n*B + b (no padding)
CH = 512         # token chunk (matmul free dim / PSUM bank)
NCH = (T + CH - 1) // CH            # 7 (last chunk short: 448)
CHS = [min(CH, T - c * CH) for c in range(NCH)]  # per-chunk sizes
KD = D // 128    # 4 contraction chunks over d
KH = H // 128    # 16 contraction chunks over h
EPS = 1e-8

# fp8 scaling (see module docstring); SV * SO must equal SH
SH = 2048.0
SG = 64.0
SV = 32.0
SO = 64.0

# Software-interleaved DoubleRow: weights pre-interleaved on host so the
# PE weight load reads contiguously instead of doing the DoubleRow
# hardware interleave read.
SWI = False

fp32 = mybir.dt.float32
bf16 = mybir.dt.bfloat16
fp8 = mybir.dt.float8e4
bf16_np = ml_dtypes.bfloat16
fp8_np = ml_dtypes.float8_e4m3
DR = mybir.MatmulPerfMode.DoubleRow

# Walrus in this toolchain rejects instructions carrying more than one
# semaphore wait; Tile's final drain aggregates many. Split extras onto
# preceding same-engine NOPs (identical sync semantics).
_MAX_WAITS = 1


def _split_excess_waits(nc, max_waits=_MAX_WAITS):
    for f in nc.m.functions:
        for bb in f.blocks:
            insts = bb.instructions
            i = 0
            while i < len(insts):
                inst = insts[i]
                si = inst.sync_info
                if si is None or si.on_wait is None or len(si.on_wait) <= max_waits:
                    i += 1
                    continue
                waits = list(si.on_wait)
                keep, extra = waits[-max_waits:], waits[:-max_waits]
                nops = []
                for j in range(0, len(extra), max_waits):
                    nops.append(
                        mybir.InstNoOp(
                            name=f"{inst.name}_ws{j}",
                            engine=inst.engine,
                            ins=[],
                            outs=[],
                            sync_info=mybir.SyncInfo(
                                on_wait=extra[j : j + max_waits], on_update=[]
                            ),
                        )
                    )
                inst.sync_info = mybir.SyncInfo(
                    on_wait=keep, on_update=list(si.on_update or [])
                )
                for k, nop in enumerate(nops):
                    insts.insert(i + k, nop)
                i += len(nops) + 1


def build_bass(split_waits=True):
    nc = bass.Bass("TRN2", target_bir_lowering=False, debug=False, num_devices=E)

    xT_d = nc.dram_tensor("xT", [KD, 128, T], fp32, kind="ExternalInput").ap()
    if SWI:
        wg_d = nc.dram_tensor(
            "wg", [L, 128, KD // 2, KH, 256], fp8, kind="ExternalInput"
        ).ap()
        wv_d = nc.dram_tensor(
            "wv", [L, 128, KD // 2, KH, 256], fp8, kind="ExternalInput"
        ).ap()
        wo_d = nc.dram_tensor(
            "wo", [L, 128, KH // 2, KD, 256], fp8, kind="ExternalInput"
        ).ap()
    else:
        wg_d = nc.dram_tensor("wg", [L, 128, KD, H], fp8, kind="ExternalInput").ap()
        wv_d = nc.dram_tensor("wv", [L, 128, KD, H], fp8, kind="ExternalInput").ap()
        wo_d = nc.dram_tensor("wo", [L, 128, KH, D], fp8, kind="ExternalInput").ap()
    wr_d = nc.dram_tensor("wr", [128, N, KD, E], bf16, kind="ExternalInput").ap()
    br_d = nc.dram_tensor("brt", [1, N * E], bf16, kind="ExternalInput").ap()
    sel_d = nc.dram_tensor("sel", [E, 1], bf16, kind="ExternalInput").ap()
    wp_d = nc.dram_tensor("wp", [128, KD, 1], bf16, kind="ExternalInput").ap()
    bp_d = nc.dram_tensor("bps", [1, 1], fp32, kind="ExternalInput").ap()
    u_d = nc.dram_tensor("u", [1, T], fp32, kind="ExternalOutput").ap()

    with tile.TileContext(nc) as tc:
        from contextlib import ExitStack

        with ExitStack() as ctx:
            const = ctx.enter_context(tc.tile_pool(name="const", bufs=1))
            hp = ctx.enter_context(tc.tile_pool(name="hpool", bufs=1))
            wpg = ctx.enter_context(tc.tile_pool(name="wpg", bufs=2))
            wpv = ctx.enter_context(tc.tile_pool(name="wpv", bufs=2))
            wpo = ctx.enter_context(tc.tile_pool(name="wpo", bufs=2))
            nrm = ctx.enter_context(tc.tile_pool(name="nrm", bufs=4))
            sqp = ctx.enter_context(tc.tile_pool(name="sqp", bufs=2))
            gvp = ctx.enter_context(tc.tile_pool(name="gvp", bufs=2))
            silup = ctx.enter_context(tc.tile_pool(name="silup", bufs=4))
            smallp = ctx.enter_context(tc.tile_pool(name="smallp", bufs=4))
            routp = ctx.enter_context(tc.tile_pool(name="routp", bufs=2))
            outp = ctx.enter_context(tc.tile_pool(name="outp", bufs=2))
            pg = ctx.enter_context(tc.tile_pool(name="pg", bufs=2, space="PSUM"))
            pv = ctx.enter_context(tc.tile_pool(name="pv", bufs=2, space="PSUM"))
            pd = ctx.enter_context(tc.tile_pool(name="pd", bufs=2, space="PSUM"))
            pm = ctx.enter_context(tc.tile_pool(name="pm", bufs=2, space="PSUM"))

            # ---- constants ----
            ones_k_bf = const.tile([128, 1], bf16, name="ones_k_bf")
            nc.vector.memset(ones_k_bf, 1.0)
            ones_m_bf = const.tile([1, 128], bf16, name="ones_m_bf")
            nc.vector.memset(ones_m_bf, 1.0)
            ones_b_bf = const.tile([1, B], bf16, name="ones_b_bf")
            nc.vector.memset(ones_b_bf, 1.0)
            ones_e_bf = const.tile([E, 1], bf16, name="ones_e_bf")
            nc.vector.memset(ones_e_bf, 1.0)

            eps_sb = const.tile([1, 1], fp32, name="eps_sb")
            nc.vector.memset(eps_sb, EPS)
            sel_sb = const.tile([E, 1], bf16, name="sel_sb")
            nc.sync.dma_start(sel_sb[:], sel_d[:])
            br_sb = const.tile([1, N * E], bf16, name="br_sb")
            nc.sync.dma_start(br_sb[:], br_d[:])
            wr_sb = const.tile([128, N, KD, E], bf16, name="wr_sb")
            nc.sync.dma_start(wr_sb[:], wr_d[:])
            wp_sb = const.tile([128, KD, 1], bf16, name="wp_sb")
            nc.sync.dma_start(wp_sb[:], wp_d[:])
            bp_sb = const.tile([1, 1], fp32, name="bp_sb")
            nc.sync.dma_start(bp_sb[:], bp_d[:])
            w_sb = const.tile([1, T], fp32, name="w_sb")  # router weight row

            # ---- residual state (fp32 at SH x true scale, dT layout) ----
            h = []
            for k in range(KD):
                hk = hp.tile([128, T], fp32, name=f"h{k}", tag=f"h{k}")
                h.append(hk)
            # per-chunk loads so chunk-0 compute starts after ~256KB, not 7MB
            for c in range(NCH):
                cs = ds(c * CH, CHS[c])
                for k in range(KD):
                    nc.sync.dma_start(h[k][:, cs], xT_d[k][:, cs])

            # ---- router: all-E logits (bf16), softmax, own weight row.
            # Emitted per chunk, interleaved with layer-0 rmsnorm so the
            # small router matmuls fill PE gaps instead of forming a
            # serial phase at kernel start.
            def emit_router(c):
                sz = CHS[c]
                cs = ds(c * CH, sz)
                xb = routp.tile([128, KD, CH], bf16, name=f"xb{c}", tag="xb")
                for k in range(KD):
                    nc.vector.tensor_copy(xb[:, k, :sz], h[k][:, cs])
                lg = pm.tile([128, CH], fp32, name=f"lg{c}", tag="pm")
                for ni in range(sz // B):
                    n = (c * CH) // B + ni
                    off = ni * B
                    for k in range(KD):
                        nc.tensor.matmul(
                            lg[0:E, ds(off, B)],
                            wr_sb[:, n, k, :],
                            xb[:, k, ds(off, B)],
                            start=(k == 0),
                            stop=False,
                        )
                    nc.tensor.matmul(
                        lg[0:E, ds(off, B)],
                        br_sb[0:1, ds(n * E, E)],
                        ones_b_bf[:],
                        start=False,
                        stop=True,
                    )
                expc = routp.tile([E, CH], bf16, name=f"expc{c}", tag="expc")
                with nc.allow_low_precision(
                    reason="bf16 exp perturbs num and den together; the ratio "
                    "keeps softmax weights to ~0.4% which is noise here"
                ):
                    nc.scalar.activation(
                        expc[:, :sz], lg[0:E, :sz], mybir.ActivationFunctionType.Exp
                    )
                den = pm.tile([128, CH], fp32, name=f"den{c}", tag="pm")
                nc.tensor.matmul(
                    den[0:1, :sz], ones_e_bf[:], expc[:, :sz], start=True, stop=True
                )
                num = pd.tile([128, CH], fp32, name=f"num{c}", tag="pd")
                nc.tensor.matmul(
                    num[0:1, :sz], sel_sb[:], expc[:, :sz], start=True, stop=True
                )
                rden = smallp.tile([1, CH], fp32, name=f"rden{c}", tag="rden")
                nc.vector.reciprocal(rden[:, :sz], den[0:1, :sz])
                nc.vector.tensor_mul(w_sb[:, cs], num[0:1, :sz], rden[:, :sz])

            # ---- expert MLP stack ----
            DRM = mybir.MatmulPerfMode.DoubleRowSwInterleave if SWI else DR
            for l in range(L):
                if SWI:
                    wg_sb = wpg.tile([128, KD // 2, KH, 256], fp8, name=f"wg{l}", tag="wg")
                    wv_sb = wpv.tile([128, KD // 2, KH, 256], fp8, name=f"wv{l}", tag="wv")
                    wo_sb = wpo.tile([128, KH // 2, KD, 256], fp8, name=f"wo{l}", tag="wo")
                else:
                    wg_sb = wpg.tile([128, KD, H], fp8, name=f"wg{l}", tag="wg")
                    wv_sb = wpv.tile([128, KD, H], fp8, name=f"wv{l}", tag="wv")
                    wo_sb = wpo.tile([128, KH, D], fp8, name=f"wo{l}", tag="wo")
                # split weight loads so the first j-blocks land early
                if SWI:
                    for g in range(4):
                        nc.sync.dma_start(
                            wg_sb[:, :, ds(4 * g, 4), :], wg_d[l][:, :, ds(4 * g, 4), :]
                        )
                        nc.sync.dma_start(
                            wv_sb[:, :, ds(4 * g, 4), :], wv_d[l][:, :, ds(4 * g, 4), :]
                        )
                        nc.sync.dma_start(
                            wo_sb[:, ds(2 * g, 2), :, :], wo_d[l][:, ds(2 * g, 2), :, :]
                        )
                else:
                    nc.sync.dma_start(wg_sb[:], wg_d[l])
                    nc.sync.dma_start(wv_sb[:], wv_d[l])
                    nc.sync.dma_start(wo_sb[:], wo_d[l])

                normed = {}
                # --- rmsnorm, emitted two chunks ahead of SwiGLU use so
                # its small PE matmuls interleave between DR bursts instead
                # of bunching (and stalling) at the layer boundary ---
                def emit_norm(c):
                    sz = CHS[c]
                    cs = ds(c * CH, sz)
                    if l == 0:
                        emit_router(c)
                    sq = sqp.tile([128, KD, CH], bf16, name=f"sq{l}_{c}", tag="sq")
                    for k in range(KD):
                        # ACT, not DVE: DVE is the busier engine in this phase
                        nc.scalar.activation(
                            sq[:, k, :sz],
                            h[k][:, cs],
                            mybir.ActivationFunctionType.Square,
                        )
                    msq = pm.tile([128, CH], fp32, name=f"ms{l}_{c}", tag="pm")
                    for k in range(KD):
                        nc.tensor.matmul(
                            msq[0:1, :sz],
                            ones_k_bf[:],
                            sq[:, k, :sz],
                            start=(k == 0),
                            stop=(k == KD - 1),
                        )
                    # std = sqrt(mean + eps); rstd = 1/std (bf16 is enough:
                    # normed is rounded to fp8 right after anyway)
                    std = smallp.tile([1, CH], fp32, name=f"std{l}_{c}", tag="std")
                    nc.scalar.activation(
                        std[:, :sz],
                        msq[0:1, :sz],
                        mybir.ActivationFunctionType.Sqrt,
                        bias=eps_sb[:],
                        scale=1.0 / D,
                    )
                    rstd = smallp.tile([1, CH], bf16, name=f"rstd{l}_{c}", tag="rstd")
                    with nc.allow_low_precision(
                        reason="rstd feeds fp8 normed; bf16 rstd is free precision-wise"
                    ):
                        nc.vector.reciprocal(rstd[:, :sz], std[:, :sz])
                    bc = pm.tile([128, CH], fp32, name=f"bc{l}_{c}", tag="pm")
                    nc.tensor.matmul(
                        bc[:, :sz], ones_m_bf[:], rstd[:, :sz], start=True, stop=True
                    )
                    nt = nrm.tile([128, KD, CH], fp8, name=f"nt{l}_{c}", tag="nt")
                    with nc.allow_low_precision(
                        reason="fp8 normed feeds the DoubleRow matmuls; unit "
                        "scale sits in e4m3's good range"
                    ):
                        for k in range(KD):
                            nc.vector.tensor_mul(nt[:, k, :sz], h[k][:, cs], bc[:, :sz])
                    normed[c] = nt

                # --- SwiGLU phase (fp8 DoubleRow matmuls) ---
                # Down-projection matmuls for chunk c are emitted during
                # chunk c+1's gate/val phase: the PE is in-order, so putting
                # a full chunk of independent gate/val work between the last
                # gv write and the down matmuls that read it removes the
                # per-chunk PE stall on the silu+mul tail.
                gv_tiles = {}

                def emit_down(c):
                    sz = CHS[c]
                    cs_ = ds(c * CH, sz)
                    gv_ = gv_tiles.pop(c)
                    if l == L - 1:
                        hbc = outp.tile([128, KD, CH], bf16, name=f"hb{c}", tag="hb")
                    for i in range(KD):
                        dps = pd.tile([128, CH], fp32, name=f"d{l}_{c}_{i}", tag="pd")
                        for j in range(KH // 2):
                            nc.tensor.matmul(
                                dps[:, :sz],
                                wo_sb[:, j, i, :] if SWI
                                else wo_sb[:, ds(2 * j, 2), ts(i, 128)],
                                gv_[:, ds(2 * j, 2), :sz],
                                start=(j == 0),
                                stop=(j == KH // 2 - 1),
                                perf_mode=DRM,
                            )
                        if l < L - 1:
                            nc.vector.tensor_add(
                                h[i][:, cs_], h[i][:, cs_], dps[:, :sz]
                            )
                        else:
                            with nc.allow_low_precision(
                                reason="final projection tolerates bf16 residual"
                            ):
                                nc.vector.tensor_add(
                                    hbc[:, i, :sz], h[i][:, cs_], dps[:, :sz]
                                )
                    if l == L - 1:
                        # final projection + router weighting for this chunk
                        eo = pm.tile([128, CH], fp32, name=f"eo{c}", tag="pm")
                        for k in range(KD):
                            nc.tensor.matmul(
                                eo[0:1, :sz],
                                wp_sb[:, k, :],
                                hbc[:, k, :sz],
                                start=(k == 0),
                                stop=(k == KD - 1),
                            )
                        eos = outp.tile([1, CH], fp32, name=f"eos{c}", tag="eos")
                        nc.scalar.activation(
                            eos[:, :sz],
                            eo[0:1, :sz],
                            mybir.ActivationFunctionType.Identity,
                            bias=bp_sb[:],
                        )
                        us = outp.tile([1, CH], fp32, name=f"us{c}", tag="us")
                        nc.vector.tensor_mul(us[:, :sz], eos[:, :sz], w_sb[:, cs_])
                        nc.sync.dma_start(u_d[0:1, cs_], us[:, :sz])

                emit_norm(0)
                if NCH > 1:
                    emit_norm(1)
                for c in range(NCH):
                    if c + 2 < NCH:
                        emit_norm(c + 2)
                    sz = CHS[c]
                    nt = normed.pop(c)
                    gv = gvp.tile([128, KH, CH], fp8, name=f"gv{l}_{c}", tag="gv")
                    gv_tiles[c] = gv
                    for j in range(KH):
                        gps = pg.tile([128, CH], fp32, name=f"g{l}_{c}_{j}", tag="pg")
                        vps = pv.tile([128, CH], fp32, name=f"v{l}_{c}_{j}", tag="pv")
                        for k in range(KD // 2):
                            nc.tensor.matmul(
                                gps[:, :sz],
                                wg_sb[:, k, j, :] if SWI
                                else wg_sb[:, ds(2 * k, 2), ts(j, 128)],
                                nt[:, ds(2 * k, 2), :sz],
                                start=(k == 0),
                                stop=(k == KD // 2 - 1),
                                perf_mode=DRM,
                            )
                        for k in range(KD // 2):
                            nc.tensor.matmul(
                                vps[:, :sz],
                                wv_sb[:, k, j, :] if SWI
                                else wv_sb[:, ds(2 * k, 2), ts(j, 128)],
                                nt[:, ds(2 * k, 2), :sz],
                                start=(k == 0),
                                stop=(k == KD // 2 - 1),
                                perf_mode=DRM,
                            )
                        sil = silup.tile([128, CH], bf16, name=f"sl{l}_{c}_{j}", tag="sil")
                        nc.scalar.activation(
                            sil[:, :sz],
                            gps[:, :sz],
                            mybir.ActivationFunctionType.Silu,
                            scale=1.0 / SG,
                        )
                        with nc.allow_low_precision(
                            reason="gv carries the val PSUM's SV factor, which "
                            "centers it in e4m3's range for the Wo matmul"
                        ):
                            nc.vector.tensor_mul(
                                gv[:, j, :sz], sil[:, :sz], vps[:, :sz]
                            )
                    if c >= 1:
                        emit_down(c - 1)
                emit_down(NCH - 1)

    if split_waits:
        _split_excess_waits(nc)
    return nc


_CACHE = {}


def _get_nc():
    if "nc" not in _CACHE:
        _CACHE["nc"] = build_bass()
    return _CACHE["nc"]


def _prep_inputs(x, scale, Wg, Wv, Wo, Wp, bp, Wr, br):
    x = np.asarray(x, np.float32)
    scale = np.asarray(scale, np.float32)
    Wg = np.asarray(Wg, np.float32)
    Wv = np.asarray(Wv, np.float32)
    Wo = np.asarray(Wo, np.float32)
    Wp = np.asarray(Wp, np.float32)
    bp = np.asarray(bp, np.float32)
    Wr = np.asarray(Wr, np.float32)
    br = np.asarray(br, np.float32)

    # xT: [d, n, b] (no padding), pre-scaled by SH -> [KD, 128, T]
    xt = x.transpose(2, 1, 0) * SH
    xT = np.ascontiguousarray(xt.reshape(KD, 128, T))

    # router tensors (shared by all cores); Wr pre-divided by SH
    wr_prep = np.ascontiguousarray(
        (Wr / SH).transpose(2, 0, 1).reshape(KD, 128, N, E).transpose(1, 2, 0, 3)
    ).astype(bf16_np)
    br_prep = np.ascontiguousarray(br.reshape(1, N * E)).astype(bf16_np)

    # fold RMSNorm scale into Wg/Wv rows: (L, E, D, H); fp8 range scales
    def q8(a):
        return np.clip(a, -240.0, 240.0).astype(fp8_np)

    wg_eff = Wg * scale[:, :, :, None] * SG
    wv_eff = Wv * scale[:, :, :, None] * SV
    wo_eff = Wo * SO

    def swi(w, kt, blocks):
        # (L, 128, kt, blocks*128) -> pre-interleaved DoubleRowSwInterleave
        # layout (L, 128, kt//2, blocks, 256): per 128-column block, pairs
        # (A[127-c], B[127-c]) of the two k-tiles, columns reversed.
        w5 = w.reshape(L, 128, kt // 2, 2, blocks, 128)
        rev = w5[..., ::-1]
        return np.ascontiguousarray(
            rev.transpose(0, 1, 2, 4, 5, 3).reshape(L, 128, kt // 2, blocks, 256)
        )

    in_maps = []
    for e in range(E):
        wg_p = q8(np.ascontiguousarray(
            wg_eff[:, e].reshape(L, KD, 128, H).transpose(0, 2, 1, 3)
        ))
        wv_p = q8(np.ascontiguousarray(
            wv_eff[:, e].reshape(L, KD, 128, H).transpose(0, 2, 1, 3)
        ))
        wo_p = q8(np.ascontiguousarray(
            wo_eff[:, e].reshape(L, KH, 128, D).transpose(0, 2, 1, 3)
        ))
        if SWI:
            wg_p = swi(wg_p, KD, KH)
            wv_p = swi(wv_p, KD, KH)
            wo_p = swi(wo_p, KH, KD)
        wp_p = np.ascontiguousarray(
            (Wp[e] / SH).reshape(KD, 128, 1).transpose(1, 0, 2)
        ).astype(bf16_np)
        sel = np.zeros((E, 1), bf16_np)
        sel[e, 0] = 1.0
        in_maps.append(
            {
                "xT": xT,
                "wg": wg_p,
                "wv": wv_p,
                "wo": wo_p,
                "wr": wr_prep,
                "brt": br_prep,
                "sel": sel,
                "wp": wp_p,
                "bps": np.array([[bp[e]]], np.float32),
            }
        )
    return in_maps


def _combine(results):
    u = np.zeros(T, np.float64)
    for r in results:
        u += r["u"].reshape(T).astype(np.float64)
    return np.ascontiguousarray(u.reshape(N, B).T).astype(np.float32)


def kernel(x, scale, Wg, Wv, Wo, Wp, bp, Wr, br):
    nc = _get_nc()
    in_maps = _prep_inputs(x, scale, Wg, Wv, Wo, Wp, bp, Wr, br)
    res = run_bass_kernel_spmd(nc, in_maps, list(range(E)))
    return _combine(res.results)


# revision 32
# speedup vs baseline: 6.1225x; 1.6940x over previous
"""Trainium2 Bass kernel for nn_MoELayer_67619965108245.

Dense MoE: B=64, N=55, D=512, E=8, L=4 SwiGLU layers per expert, H=2048.
Expert-parallel over 8 NeuronCores (one expert per core).

Layout: all activations live transposed in SBUF as [d_model, tokens]
("dT layout", tokens n-major: t = n*64 + b, N padded 55->56 so T=3584).
This makes every matmul in the SwiGLU chain transpose-free:
  gate^T[h,t] = sum_d Wg[d,h] * normed^T[d,t]      (lhsT = Wg natural)
  delta^T[d,t] = sum_h Wo[h,d] * gv^T[h,t]         (lhsT = Wo natural)
RMSNorm reductions over d (the partition dim) are done with ones-vector
matmuls on the PE; the per-token rstd row is broadcast back across
partitions with a K=1 ones matmul.

The three big SwiGLU matmuls run in fp8-e4m3 with perf_mode=DoubleRow
(2 fp8 weights per PE cell, 2 MACs/cycle -> ~2x bf16 matmul
throughput). Scaling keeps everything in e4m3's good range with zero
extra instructions: x is pre-scaled by SH=2048 on host and the residual
stream h stays at 2048x true scale all 4 layers (RMSNorm is
scale-invariant, so normed is always unit-scale); Wg*SG, Wv*SV, Wo*SO
are folded on host with SV*SO == SH so the Wo PSUM lands back at h's
scale and the residual add needs no rescale. The silu reads its PSUM
with scale=1/SG; the gv product keeps the val PSUM's SV factor as its
fp8 range boost. Wr and Wp are pre-divided by SH on host.

Each core computes its expert's full stack plus the complete router
softmax (all 8 logits per token, in bf16), and returns
u_e[t] = expert_out_e[t] * softmax_weight_e[t]. Host just sums the 8
partial vectors. The RMSNorm scale is folded into Wg/Wv rows on host.
"""

import numpy as np
import ml_dtypes

import concourse.bass as bass
import concourse.tile as tile
import concourse.mybir as mybir
from concourse.bass import ds, ts
from concourse.bass_utils import run_bass_kernel_spmd

B, N, D, E, L = 64, 55, 512, 8, 4
H = 4 * D
T = N * B        # 3520 tokens, t = n*B + b (no padding)
CH = 512         # token chunk (matmul free dim / PSUM bank)
NCH = (T + CH - 1) // CH            # 7 (last chunk short: 448)
CHS = [min(CH, T - c * CH) for c in range(NCH)]  # per-chunk sizes
KD = D // 128    # 4 contraction chunks over d
KH = H // 128    # 16 contraction chunks over h
EPS = 1e-8

# fp8 scaling (see module docstring); SV * SO must equal SH
SH = 2048.0
SG = 64.0
SV = 32.0
SO = 64.0

# Software-interleaved DoubleRow: weights pre-interleaved on host so the
# PE weight load reads contiguously instead of doing the DoubleRow
# hardware interleave read.
SWI = False

fp32 = mybir.dt.float32
bf16 = mybir.dt.bfloat16
fp8 = mybir.dt.float8e4
bf16_np = ml_dtypes.bfloat16
fp8_np = ml_dtypes.float8_e4m3
DR = mybir.MatmulPerfMode.DoubleRow

# Walrus in this toolchain rejects instructions carrying more than one
# semaphore wait; Tile's final drain aggregates many. Split extras onto
# preceding same-engine NOPs (identical sync semantics).
_MAX_WAITS = 1


def _split_excess_waits(nc, max_waits=_MAX_WAITS):
    for f in nc.m.functions:
        for bb in f.blocks:
            insts = bb.instructions
            i = 0
            while i < len(insts):
                inst = insts[i]
                si = inst.sync_info
                if si is None or si.on_wait is None or len(si.on_wait) <= max_waits:
                    i += 1
                    continue
                waits = list(si.on_wait)
                keep, extra = waits[-max_waits:], waits[:-max_waits]
                nops = []
                for j in range(0, len(extra), max_waits):
                    nops.append(
                        mybir.InstNoOp(
                            name=f"{inst.name}_ws{j}",
                            engine=inst.engine,
                            ins=[],
                            outs=[],
                            sync_info=mybir.SyncInfo(
                                on_wait=extra[j : j + max_waits], on_update=[]
                            ),
                        )
                    )
                inst.sync_info = mybir.SyncInfo(
                    on_wait=keep, on_update=list(si.on_update or [])
                )
                for k, nop in enumerate(nops):
                    insts.insert(i + k, nop)
                i += len(nops) + 1


def build_bass(split_waits=True):
    nc = bass.Bass("TRN2", target_bir_lowering=False, debug=False, num_devices=E)

    xT_d = nc.dram_tensor("xT", [KD, 128, T], fp32, kind="ExternalInput").ap()
    if SWI:
        wg_d = nc.dram_tensor(
            "wg", [L, 128, KD // 2, KH, 256], fp8, kind="ExternalInput"
        ).ap()
        wv_d = nc.dram_tensor(
            "wv", [L, 128, KD // 2, KH, 256], fp8, kind="ExternalInput"
        ).ap()
        wo_d = nc.dram_tensor(
            "wo", [L, 128, KH // 2, KD, 256], fp8, kind="ExternalInput"
        ).ap()
    else:
        wg_d = nc.dram_tensor("wg", [L, 128, KD, H], fp8, kind="ExternalInput").ap()
        wv_d = nc.dram_tensor("wv", [L, 128, KD, H], fp8, kind="ExternalInput").ap()
        wo_d = nc.dram_tensor("wo", [L, 128, KH, D], fp8, kind="ExternalInput").ap()
    wr_d = nc.dram_tensor("wr", [128, N, KD, E], bf16, kind="ExternalInput").ap()
    br_d = nc.dram_tensor("brt", [1, N * E], bf16, kind="ExternalInput").ap()
    sel_d = nc.dram_tensor("sel", [E, 1], bf16, kind="ExternalInput").ap()
    wp_d = nc.dram_tensor("wp", [128, KD, 1], bf16, kind="ExternalInput").ap()
    bp_d = nc.dram_tensor("bps", [1, 1], fp32, kind="ExternalInput").ap()
    u_d = nc.dram_tensor("u", [1, T], fp32, kind="ExternalOutput").ap()

    with tile.TileContext(nc) as tc:
        from contextlib import ExitStack

        with ExitStack() as ctx:
            const = ctx.enter_context(tc.tile_pool(name="const", bufs=1))
            hp = ctx.enter_context(tc.tile_pool(name="hpool", bufs=1))
            wpg = ctx.enter_context(tc.tile_pool(name="wpg", bufs=2))
            wpv = ctx.enter_context(tc.tile_pool(name="wpv", bufs=2))
            wpo = ctx.enter_context(tc.tile_pool(name="wpo", bufs=2))
            nrm = ctx.enter_context(tc.tile_pool(name="nrm", bufs=4))
            sqp = ctx.enter_context(tc.tile_pool(name="sqp", bufs=2))
            gvp = ctx.enter_context(tc.tile_pool(name="gvp", bufs=2))
            silup = ctx.enter_context(tc.tile_pool(name="silup", bufs=4))
            smallp = ctx.enter_context(tc.tile_pool(name="smallp", bufs=4))
            routp = ctx.enter_context(tc.tile_pool(name="routp", bufs=2))
            outp = ctx.enter_context(tc.tile_pool(name="outp", bufs=2))
            pg = ctx.enter_context(tc.tile_pool(name="pg", bufs=2, space="PSUM"))
            pv = ctx.enter_context(tc.tile_pool(name="pv", bufs=2, space="PSUM"))
            pd = ctx.enter_context(tc.tile_pool(name="pd", bufs=2, space="PSUM"))
            pm = ctx.enter_context(tc.tile_pool(name="pm", bufs=2, space="PSUM"))

            # ---- constants ----
            ones_k_bf = const.tile([128, 1], bf16, name="ones_k_bf")
            nc.vector.memset(ones_k_bf, 1.0)
            ones_m_bf = const.tile([1, 128], bf16, name="ones_m_bf")
            nc.vector.memset(ones_m_bf, 1.0)
            ones_b_bf = const.tile([1, B], bf16, name="ones_b_bf")
            nc.vector.memset(ones_b_bf, 1.0)
            ones_e_bf = const.tile([E, 1], bf16, name="ones_e_bf")
            nc.vector.memset(ones_e_bf, 1.0)

            eps_sb = const.tile([1, 1], fp32, name="eps_sb")
            nc.vector.memset(eps_sb, EPS)
            sel_sb = const.tile([E, 1], bf16, name="sel_sb")
            nc.sync.dma_start(sel_sb[:], sel_d[:])
            br_sb = const.tile([1, N * E], bf16, name="br_sb")
            nc.sync.dma_start(br_sb[:], br_d[:])
            wr_sb = const.tile([128, N, KD, E], bf16, name="wr_sb")
            nc.sync.dma_start(wr_sb[:], wr_d[:])
            wp_sb = const.tile([128, KD, 1], bf16, name="wp_sb")
            nc.sync.dma_start(wp_sb[:], wp_d[:])
            bp_sb = const.tile([1, 1], fp32, name="bp_sb")
            nc.sync.dma_start(bp_sb[:], bp_d[:])
            w_sb = const.tile([1, T], fp32, name="w_sb")  # router weight row

            # ---- residual state (fp32 at SH x true scale, dT layout) ----
            h = []
            for k in range(KD):
                hk = hp.tile([128, T], fp32, name=f"h{k}", tag=f"h{k}")
                h.append(hk)
            # per-chunk loads so chunk-0 compute starts after ~256KB, not 7MB
            for c in range(NCH):
                cs = ds(c * CH, CHS[c])
                for k in range(KD):
                    nc.sync.dma_start(h[k][:, cs], xT_d[k][:, cs])

            # ---- router: all-E logits (bf16), softmax, own weight row.
            # Emitted per chunk, interleaved with layer-0 rmsnorm so the
            # small router matmuls fill PE gaps instead of forming a
            # serial phase at kernel start.
            def emit_router(c):
                sz = CHS[c]
                cs = ds(c * CH, sz)
                xb = routp.tile([128, KD, CH], bf16, name=f"xb{c}", tag="xb")
                for k in range(KD):
                    nc.vector.tensor_copy(xb[:, k, :sz], h[k][:, cs])
                lg = pm.tile([128, CH], fp32, name=f"lg{c}", tag="pm")
                for ni in range(sz // B):
                    n = (c * CH) // B + ni
                    off = ni * B
                    for k in range(KD):
                        nc.tensor.matmul(
                            lg[0:E, ds(off, B)],
                            wr_sb[:, n, k, :],
                            xb[:, k, ds(off, B)],
                            start=(k == 0),
                            stop=False,
                        )
                    nc.tensor.matmul(
                        lg[0:E, ds(off, B)],
                        br_sb[0:1, ds(n * E, E)],
                        ones_b_bf[:],
                        start=False,
                        stop=True,
                    )
                expc = routp.tile([E, CH], bf16, name=f"expc{c}", tag="expc")
                with nc.allow_low_precision(
                    reason="bf16 exp perturbs num and den together; the ratio "
                    "keeps softmax weights to ~0.4% which is noise here"
                ):
                    nc.scalar.activation(
                        expc[:, :sz], lg[0:E, :sz], mybir.ActivationFunctionType.Exp
                    )
                den = pm.tile([128, CH], fp32, name=f"den{c}", tag="pm")
                nc.tensor.matmul(
                    den[0:1, :sz], ones_e_bf[:], expc[:, :sz], start=True, stop=True
                )
                num = pd.tile([128, CH], fp32, name=f"num{c}", tag="pd")
                nc.tensor.matmul(
                    num[0:1, :sz], sel_sb[:], expc[:, :sz], start=True, stop=True
                )
                rden = smallp.tile([1, CH], fp32, name=f"rden{c}", tag="rden")
                nc.vector.reciprocal(rden[:, :sz], den[0:1, :sz])
                nc.vector.tensor_mul(w_sb[:, cs], num[0:1, :sz], rden[:, :sz])

            # ---- expert MLP stack ----
            DRM = mybir.MatmulPerfMode.DoubleRowSwInterleave if SWI else DR
            for l in range(L):
                if SWI:
                    wg_sb = wpg.tile([128, KD // 2, KH, 256], fp8, name=f"wg{l}", tag="wg")
                    wv_sb = wpv.tile([128, KD // 2, KH, 256], fp8, name=f"wv{l}", tag="wv")
                    wo_sb = wpo.tile([128, KH // 2, KD, 256], fp8, name=f"wo{l}", tag="wo")
                else:
                    wg_sb = wpg.tile([128, KD, H], fp8, name=f"wg{l}", tag="wg")
                    wv_sb = wpv.tile([128, KD, H], fp8, name=f"wv{l}", tag="wv")
                    wo_sb = wpo.tile([128, KH, D], fp8, name=f"wo{l}", tag="wo")
                # split weight loads so the first j-blocks land early
                if SWI:
                    for g in range(4):
                        nc.sync.dma_start(
                            wg_sb[:, :, ds(4 * g, 4), :], wg_d[l][:, :, ds(4 * g, 4), :]
                        )
                        nc.sync.dma_start(
                            wv_sb[:, :, ds(4 * g, 4), :], wv_d[l][:, :, ds(4 * g, 4), :]
                        )
                        nc.sync.dma_start(
                            wo_sb[:, ds(2 * g, 2), :, :], wo_d[l][:, ds(2 * g, 2), :, :]
                        )
                else:
                    nc.sync.dma_start(wg_sb[:], wg_d[l])
                    nc.sync.dma_start(wv_sb[:], wv_d[l])
                    nc.sync.dma_start(wo_sb[:], wo_d[l])

                normed = {}
                # --- rmsnorm, emitted two chunks ahead of SwiGLU use so
                # its small PE matmuls interleave between DR bursts instead
                # of bunching (and stalling) at the layer boundary ---
                def emit_norm(c):
                    sz = CHS[c]
                    cs = ds(c * CH, sz)
                    if l == 0:
                        emit_router(c)
                    sq = sqp.tile([128, KD, CH], bf16, name=f"sq{l}_{c}", tag="sq")
                    for k in range(KD):
                        # ACT, not DVE: DVE is the busier engine in this phase
                        nc.scalar.activation(
                            sq[:, k, :sz],
                            h[k][:, cs],
                            mybir.ActivationFunctionType.Square,
                        )
                    msq = pm.tile([128, CH], fp32, name=f"ms{l}_{c}", tag="pm")
                    for k in range(KD):
                        nc.tensor.matmul(
                            msq[0:1, :sz],
                            ones_k_bf[:],
                            sq[:, k, :sz],
                            start=(k == 0),
                            stop=(k == KD - 1),
                        )
                    # std = sqrt(mean + eps); rstd = 1/std (bf16 is enough:
                    # normed is rounded to fp8 right after anyway)
                    std = smallp.tile([1, CH], fp32, name=f"std{l}_{c}", tag="std")
                    nc.scalar.activation(
                        std[:, :sz],
                        msq[0:1, :sz],
                        mybir.ActivationFunctionType.Sqrt,
                        bias=eps_sb[:],
                        scale=1.0 / D,
                    )
                    rstd = smallp.tile([1, CH], bf16, name=f"rstd{l}_{c}", tag="rstd")
                    with nc.allow_low_precision(
                        reason="rstd feeds fp8 normed; bf16 rstd is free precision-wise"
                    ):
                        nc.vector.reciprocal(rstd[:, :sz], std[:, :sz])
                    bc = pm.tile([128, CH], fp32, name=f"bc{l}_{c}", tag="pm")
                    nc.tensor.matmul(
                        bc[:, :sz], ones_m_bf[:], rstd[:, :sz], start=True, stop=True
                    )
                    nt = nrm.tile([128, KD, CH], fp8, name=f"nt{l}_{c}", tag="nt")
                    with nc.allow_low_precision(
                        reason="fp8 normed feeds the DoubleRow matmuls; unit "
                        "scale sits in e4m3's good range"
                    ):
                        for k in range(KD):
                            nc.vector.tensor_mul(nt[:, k, :sz], h[k][:, cs], bc[:, :sz])
                    normed[c] = nt

                # --- SwiGLU phase (fp8 DoubleRow matmuls) ---
                # Down-projection matmuls for chunk c are emitted during
                # chunk c+1's gate/val phase: the PE is in-order, so putting
                # a full chunk of independent gate/val work between the last
                # gv write and the down matmuls that read it removes the
                # per-chunk PE stall on the silu+mul tail.
                gv_tiles = {}

                def emit_down(c):
                    sz = CHS[c]
                    cs_ = ds(c * CH, sz)
                    gv_ = gv_tiles.pop(c)
                    if l == L - 1:
                        hbc = outp.tile([128, KD, CH], bf16, name=f"hb{c}", tag="hb")
                    for i in range(KD):
                        dps = pd.tile([128, CH], fp32, name=f"d{l}_{c}_{i}", tag="pd")
                        for j in range(KH // 2):
                            nc.tensor.matmul(
                                dps[:, :sz],
                                wo_sb[:, j, i, :] if SWI
                                else wo_sb[:, ds(2 * j, 2), ts(i, 128)],
                                gv_[:, ds(2 * j, 2), :sz],
                                start=(j == 0),
                                stop=(j == KH // 2 - 1),
                                perf_mode=DRM,
                            )
                        if l < L - 1:
                            nc.vector.tensor_add(
                                h[i][:, cs_], h[i][:, cs_], dps[:, :sz]
                            )
                        else:
                            with nc.allow_low_precision(
                                reason="final projection tolerates bf16 residual"
                            ):
                                nc.vector.tensor_add(
                                    hbc[:, i, :sz], h[i][:, cs_], dps[:, :sz]
                                )
                    if l == L - 1:
                        # final projection + router weighting for this chunk
                        eo = pm.tile([128, CH], fp32, name=f"eo{c}", tag="pm")
                        for k in range(KD):
                            nc.tensor.matmul(
                                eo[0:1, :sz],
                                wp_sb[:, k, :],
                                hbc[:, k, :sz],
                                start=(k == 0),
                                stop=(k == KD - 1),
                            )
                        eos = outp.tile([1, CH], fp32, name=f"eos{c}", tag="eos")
                        nc.scalar.activation(
                            eos[:, :sz],
                            eo[0:1, :sz],
                            mybir.ActivationFunctionType.Identity,
                            bias=bp_sb[:],
                        )
                        us = outp.tile([1, CH], fp32, name=f"us{c}", tag="us")
                        nc.vector.tensor_mul(us[:, :sz], eos[:, :sz], w_sb[:, cs_])
                        nc.sync.dma_start(u_d[0:1, cs_], us[:, :sz])

                emit_norm(0)
                if NCH > 1:
                    emit_norm(1)
                for c in range(NCH):
                    if c + 2 < NCH:
                        emit_norm(c + 2)
                    sz = CHS[c]
                    nt = normed.pop(c)
                    gv = gvp.tile([128, KH, CH], fp8, name=f"gv{l}_{c}", tag="gv")
                    gv_tiles[c] = gv
                    for j in range(KH):
                        gps = pg.tile([128, CH], fp32, name=f"g{l}_{c}_{j}", tag="pg")
                        vps = pv.tile([128, CH], fp32, name=f"v{l}_{c}_{j}", tag="pv")
                        for k in range(KD // 2):
                            nc.tensor.matmul(
                                gps[:, :sz],
                                wg_sb[:, k, j, :] if SWI
                                else wg_sb[:, ds(2 * k, 2), ts(j, 128)],
                                nt[:, ds(2 * k, 2), :sz],
                                start=(k == 0),
                                stop=(k == KD // 2 - 1),
                                perf_mode=DRM,
                            )
                        for k in range(KD // 2):
                            nc.tensor.matmul(
                                vps[:, :sz],
                                wv_sb[:, k, j, :] if SWI
                                else wv_sb[:, ds(2 * k, 2), ts(j, 128)],
                                nt[:, ds(2 * k, 2), :sz],
                                start=(k == 0),
                                stop=(k == KD // 2 - 1),
                                perf_mode=DRM,
                            )
                        sil = silup.tile([128, CH], bf16, name=f"sl{l}_{c}_{j}", tag="sil")
                        nc.scalar.activation(
                            sil[:, :sz],
                            gps[:, :sz],
                            mybir.ActivationFunctionType.Silu,
                            scale=1.0 / SG,
                        )
                        with nc.allow_low_precision(
                            reason="gv carries the val PSUM's SV factor, which "
                            "centers it in e4m3's range for the Wo matmul"
                        ):
                            nc.vector.tensor_mul(
                                gv[:, j, :sz], sil[:, :sz], vps[:, :sz]
                            )
                    if c >= 1:
                        emit_down(c - 1)
                emit_down(NCH - 1)

    if split_waits:
        _split_excess_waits(nc)
    return nc


_CACHE = {}


def _get_nc():
    if "nc" not in _CACHE:
        _CACHE["nc"] = build_bass()
    return _CACHE["nc"]


def _prep_inputs(x, scale, Wg, Wv, Wo, Wp, bp, Wr, br):
    x = np.asarray(x, np.float32)
    scale = np.asarray(scale, np.float32)
    Wg = np.asarray(Wg, np.float32)
    Wv = np.asarray(Wv, np.float32)
    Wo = np.asarray(Wo, np.float32)
    Wp = np.asarray(Wp, np.float32)
    bp = np.asarray(bp, np.float32)
    Wr = np.asarray(Wr, np.float32)
    br = np.asarray(br, np.float32)

    # xT: [d, n, b] (no padding), pre-scaled by SH -> [KD, 128, T]
    xt = x.transpose(2, 1, 0) * SH
    xT = np.ascontiguousarray(xt.reshape(KD, 128, T))

    # router tensors (shared by all cores); Wr pre-divided by SH
    wr_prep = np.ascontiguousarray(
        (Wr / SH).transpose(2, 0, 1).reshape(KD, 128, N, E).transpose(1, 2, 0, 3)
    ).astype(bf16_np)
    br_prep = np.ascontiguousarray(br.reshape(1, N * E)).astype(bf16_np)

    # fold RMSNorm scale into Wg/Wv rows: (L, E, D, H); fp8 range scales
    def q8(a):
        return np.clip(a, -240.0, 240.0).astype(fp8_np)

    wg_eff = Wg * scale[:, :, :, None] * SG
    wv_eff = Wv * scale[:, :, :, None] * SV
    wo_eff = Wo * SO

    def swi(w, kt, blocks):
        # (L, 128, kt, blocks*128) -> pre-interleaved DoubleRowSwInterleave
        # layout (L, 128, kt//2, blocks, 256): per 128-column block, pairs
        # (A[127-c], B[127-c]) of the two k-tiles, columns reversed.
        w5 = w.reshape(L, 128, kt // 2, 2, blocks, 128)
        rev = w5[..., ::-1]
        return np.ascontiguousarray(
            rev.transpose(0, 1, 2, 4, 5, 3).reshape(L, 128, kt // 2, blocks, 256)
        )

    in_maps = []
    for e in range(E):
        wg_p = q8(np.ascontiguousarray(
            wg_eff[:, e].reshape(L, KD, 128, H).transpose(0, 2, 1, 3)
        ))
        wv_p = q8(np.ascontiguousarray(
            wv_eff[:, e].reshape(L, KD, 128, H).transpose(0, 2, 1, 3)
        ))
        wo_p = q8(np.ascontiguousarray(
            wo_eff[:, e].reshape(L, KH, 128, D).transpose(0, 2, 1, 3)
        ))
        if SWI:
            wg_p = swi(wg_p, KD, KH)
            wv_p = swi(wv_p, KD, KH)
            wo_p = swi(wo_p, KH, KD)
        wp_p = np.ascontiguousarray(
            (Wp[e] / SH).reshape(KD, 128, 1).transpose(1, 0, 2)
        ).astype(bf16_np)
        sel = np.zeros((E, 1), bf16_np)
        sel[e, 0] = 1.0
        in_maps.append(
            {
                "xT": xT,
                "wg": wg_p,
                "wv": wv_p,
                "wo": wo_p,
                "wr": wr_prep,
                "brt": br_prep,
                "sel": sel,
                "wp": wp_p,
                "bps": np.array([[bp[e]]], np.float32),
            }
        )
    return in_maps


def _combine(results):
    u = np.zeros(T, np.float64)
    for r in results:
        u += r["u"].reshape(T).astype(np.float64)
    return np.ascontiguousarray(u.reshape(N, B).T).astype(np.float32)


def kernel(x, scale, Wg, Wv, Wo, Wp, bp, Wr, br):
    nc = _get_nc()
    in_maps = _prep_inputs(x, scale, Wg, Wv, Wo, Wp, bp, Wr, br)
    res = run_bass_kernel_spmd(nc, in_maps, list(range(E)))
    return _combine(res.results)
